# revision 20
# baseline (speedup 1.0000x reference)
"""XNOR-Net conv2d kernel for Trainium2.

Computes conv2d(sign(x), sign(W), stride=1, pad=1) * alpha for
x:(32,256,56,56) f32, W:(256,256,3,3) f32, alpha:(256,1,1) f32.

Strategy: data-parallel over batch (4 images per core x 8 cores).
Per core, implicit GEMM on the PE array in fp8. sign(x) is +-1 in
fp8e4 (exact); sign(W) is represented as +-0.5 (one-pass DVE compute:
(w>0) - 0.5), with the missing x2 folded into alpha. Products are
+-0.5, accumulated in fp32 PSUM -> half-integers, exact; the final
scale restores integers, so the result is bit-exact vs the reference.

sign(x) lives in SBUF as a zero-padded fp8 image
[128 part = C_in%128, 2 c-groups, 58 rows, 64 row-stride]. Each 3x3
tap is one DoubleRow matmul contracting all 256 input channels
(K = 128 partitions x 2 c-groups): lhsT [128, 2cg, 128co], rhs
[128, 2cg, 8 rows, 56 cols] (shifted window, N=448). 9 taps
accumulate into one PSUM bank; copyback applies 2*alpha.

v2 schedule (vs v1 baseline at ~128us):
- PE prewarm: ~16 dummy N=512 matmuls on zeros from t~0 so the HAM
  clock gate reaches 8/8 before the first real matmul (real matmuls
  previously ran at 1.2 GHz until t~21us).
- Startup: weight DMA for output-half 0 is issued first, x chunk 0
  right after; transposes feed [128,3,128] PSUM tiles so the
  PSUM->SBUF casts batch 3 taps per DVE op.
- y stores go out on the scalar-engine DMA queue, one store per
  (img, h0) covering both output halves, so stores never queue behind
  the next image's x loads on the sync queue (this serialized PE at
  every image boundary via output-buffer reuse).
- x loads come in as 16-row DMAs (8 per image instead of 14).
"""

import sys

sys.path.insert(0, "/opt/trn_rl_repo")

import numpy as np

import concourse.bass as bass
import concourse.mybir as mybir
from concourse import bacc
from concourse.bass_utils import run_bass_kernel_spmd
from concourse.masks import make_identity
from concourse.tile import TileContext

P = 128
N_CORES = 8
N_IMG = 32
IMG_PER_CORE = N_IMG // N_CORES
C = 256
H = W = 56
HP = 58  # padded rows (0..57)
WS = 64  # row stride of padded buffer (cols 0..57 used, 58+ never read)
CHUNK = 8  # output rows per matmul tile -> N = 8*56 = 448
LCHUNK = 16  # max rows per x load DMA
# (row0, nrows) per load DMA: a short first chunk so the first matmul
# group's rows (x rows 0..8) land and sign in one op as early as possible
CHUNKS = [(0, 9), (9, 16), (25, 16), (41, 15)]
FP8 = mybir.dt.float8e4
N_WARM = 12  # dummy matmuls to lift the HAM clock gate before real work

last_result = None  # stash of BassKernelResults for test harnesses


def build_conv_kernel():
    nc = bacc.Bacc()
    x_in = nc.declare_dram_parameter(
        "x", [IMG_PER_CORE, C, H, W], mybir.dt.float32, isOutput=False
    )
    w_in = nc.declare_dram_parameter("w", [C, C, 3, 3], mybir.dt.float32, isOutput=False)
    a_in = nc.declare_dram_parameter("alpha", [C, 1, 1], mybir.dt.float32, isOutput=False)
    y_out = nc.declare_dram_parameter(
        "y", [IMG_PER_CORE, C, H, W], mybir.dt.float32, isOutput=True
    )
    x_ap, w_ap, a_ap, y_ap = x_in[:], w_in[:], a_in[:], y_out[:]

    with TileContext(nc) as tc:
        with (
            tc.tile_pool(name="wpool", bufs=1) as wpool,
            tc.tile_pool(name="xpool", bufs=3) as xpool,
            tc.tile_pool(name="opool", bufs=8) as opool,
            tc.tile_pool(name="pp", bufs=4, space="PSUM") as pp,
        ):
            # PE prewarm: matmuls over zeros, issued before any real
            # dependency, so the clock gate is at 8/8 when weights land
            warm_rhs = wpool.tile([P, 512], FP8, name="warm_rhs")
            nc.vector.memset(warm_rhs, 0.0)
            warm_acc = pp.tile([P, 512], mybir.dt.float32, name="warm_acc", bufs=1)

            def emit_warm(n):
                for _ in range(n):
                    nc.tensor.matmul(
                        warm_acc, warm_rhs[:, 0:P], warm_rhs, start=True, stop=True
                    )

            emit_warm(9)

            # warm up the ACT function table while the first DMAs run
            warm = wpool.tile([P, 1], mybir.dt.float32, name="warm")
            nc.vector.memset(warm, 0.0)
            nc.scalar.sign(warm, warm)

            ident = wpool.tile([P, P], mybir.dt.bfloat16, name="ident")
            make_identity(nc, ident)
            alpha_sb = wpool.tile([P, 2], mybir.dt.float32, name="alpha_sb")
            # alpha rides the scalar-engine queue: the sync queue's head is
            # the startup critical path (weights + first x rows)
            nc.scalar.dma_start(
                out=alpha_sb, in_=a_ap.flatten().rearrange("(mt co) -> co mt", co=P)
            )

            # [ci_lo, cg, mt, pos, co]
            w_lhsT = wpool.tile([P, 2, 2, 9, P], FP8, name="w_lhsT")

            wsrcs = {}
            wsgns = {}

            def emit_wdma(mt):
                for cg in range(2):
                    wsrc = wpool.tile(
                        [P, P, 9], mybir.dt.float32, name=f"wsrc{mt}", bufs=2
                    )
                    nc.sync.dma_start(
                        out=wsrc,
                        in_=w_ap[
                            mt * P : (mt + 1) * P, cg * P : (cg + 1) * P
                        ].rearrange("co ci kh kw -> co ci (kh kw)"),
                    )
                    wsrcs[(mt, cg)] = wsrc

            def emit_wsigns(mt):
                # one-pass half-sign on DVE: (w > 0) - 0.5 -> +-0.5
                for cg in range(2):
                    wsgn = wpool.tile(
                        [P, P, 9], mybir.dt.bfloat16, name=f"wsgn{mt}", bufs=2
                    )
                    nc.vector.tensor_scalar(
                        out=wsgn,
                        in0=wsrcs[(mt, cg)],
                        scalar1=0.0,
                        scalar2=0.5,
                        op0=mybir.AluOpType.is_gt,
                        op1=mybir.AluOpType.subtract,
                    )
                    wsgns[(mt, cg)] = wsgn

            def emit_wtrans(mt, cg, tri, cast_on_scalar=False):
                # transpose taps 3*tri..3*tri+2 into one PSUM tile, then a
                # single cast moves all three into the fp8 lhsT; during
                # startup half the casts go to the idle scalar engine so the
                # DVE cast chain doesn't gate the first matmuls
                tp = pp.tile([P, 3, P], mybir.dt.bfloat16, name="tp", bufs=3)
                for k in range(3):
                    nc.tensor.transpose(
                        tp[:, k, :], wsgns[(mt, cg)][:, :, 3 * tri + k], ident
                    )
                dst = w_lhsT[:, cg, mt, 3 * tri : 3 * tri + 3, :]
                if cast_on_scalar:
                    nc.scalar.copy(out=dst, in_=tp)
                else:
                    nc.vector.tensor_copy(out=dst, in_=tp)

            xpads = {}

            def emit_xpad(img):
                xpad = xpool.tile([P, 2, HP, WS], FP8, name="xpad")
                xpads[img] = xpad
                nc.vector.memset(xpad[:, :, 0, 0:58], 0.0)
                nc.vector.memset(xpad[:, :, HP - 1, 0:58], 0.0)
                nc.vector.memset(xpad[:, :, 1 : HP - 1, 0], 0.0)
                nc.vector.memset(xpad[:, :, 1 : HP - 1, 57], 0.0)

            def emit_loads(img, chunks=None):
                if chunks is None:
                    emit_xpad(img)
                    chunks = CHUNKS
                srcs = []
                for r0, rows in chunks:
                    for cg in range(2):
                        xsrc = xpool.tile(
                            [P, LCHUNK, W], mybir.dt.float32, name="xsrc", bufs=10
                        )
                        nc.sync.dma_start(
                            out=xsrc[:, 0:rows, :],
                            in_=x_ap[img, cg * P : (cg + 1) * P, r0 : r0 + rows],
                        )
                        srcs.append((r0, rows, cg, xsrc))
                return srcs

            def emit_signs(img, srcs):
                xpad = xpads[img]
                for r0, rows, cg, xsrc in srcs:
                    nc.scalar.sign(
                        xpad[:, cg, r0 + 1 : r0 + 1 + rows, 1 : W + 1],
                        xsrc[:, 0:rows, :],
                    )

            def emit_mm_group(img, h0, mt, ot, split_cb=False):
                xpad = xpads[img]
                acc = pp.tile([P, CHUNK * W], mybir.dt.float32, name="acc")
                k = 0
                for kh in range(3):
                    for kw in range(3):
                        nc.tensor.matmul(
                            acc,
                            w_lhsT[:, :, mt, kh * 3 + kw, :],
                            xpad[:, :, h0 + kh : h0 + kh + CHUNK, kw : kw + W],
                            start=(k == 0),
                            stop=(k == 8),
                            perf_mode=mybir.MatmulPerfMode.DoubleRow,
                        )
                        k += 1
                # x2 restores the +-0.5 weight scale, fused here so no alpha
                # pre-scale op ever sits at the DVE queue head during startup
                # (split_cb: halve the very last copyback so the kernel-tail
                # flush after the final matmul is shorter)
                pieces = [(0, 4), (4, 4)] if split_cb else [(0, CHUNK)]
                for r0, nr in pieces:
                    nc.vector.tensor_scalar(
                        out=ot[:, mt, r0 : r0 + nr],
                        in0=acc.rearrange("p (r c) -> p r c", c=W)[:, r0 : r0 + nr],
                        scalar1=alpha_sb[:, mt : mt + 1],
                        scalar2=2.0,
                        op0=mybir.AluOpType.mult,
                        op1=mybir.AluOpType.mult,
                    )

            def emit_row_group(img, h0, split_store=False):
                # both output halves for rows h0..h0+8, then one store on
                # the scalar-engine DMA queue (separate from x loads);
                # split_store issues per-half stores so the final flush
                # after the last matmul is shorter
                ot = opool.tile([P, 2, CHUNK, W], mybir.dt.float32, name="ot")
                ydst = y_ap[img].rearrange("(mt c) h w -> c mt h w", mt=2)[
                    :, :, h0 : h0 + CHUNK, :
                ]
                emit_mm_group(img, h0, 0, ot)
                if split_store:
                    nc.scalar.dma_start(out=ydst[:, 0:1], in_=ot[:, 0:1])
                emit_mm_group(img, h0, 1, ot, split_cb=split_store)
                if split_store:
                    nc.scalar.dma_start(out=ydst[:, 1:2, 0:4], in_=ot[:, 1:2, 0:4])
                    nc.scalar.dma_start(out=ydst[:, 1:2, 4:8], in_=ot[:, 1:2, 4:8])
                else:
                    nc.scalar.dma_start(out=ydst, in_=ot)

            def emit_mms(img):
                for h0 in range(0, H, CHUNK):
                    emit_row_group(
                        img,
                        h0,
                        split_store=(img == IMG_PER_CORE - 1 and h0 == H - CHUNK),
                    )

            # startup: sync-queue order is the critical path -- weight half
            # 0 first, then the first x rows, then weight half 1, then the
            # rest; transposes for half 0 are emitted immediately so they
            # run as soon as the signs land (behind the prewarm matmuls)
            # highest priority: the mt0 weight DMAs + signs gate the longest
            # prep chain (sign -> transpose -> cast); without this the tile
            # scheduler lets the x loads starve them on the sync queue
            with tc.high_priority():
                emit_wdma(0)
                emit_wsigns(0)
            emit_xpad(0)
            srcs0 = emit_loads(0, chunks=CHUNKS[:2])
            emit_signs(0, srcs0)
            for tri in range(3):
                emit_wtrans(0, 0, tri, cast_on_scalar=True)
            emit_warm(2)
            for tri in range(3):
                emit_wtrans(0, 1, tri)
            emit_warm(2)
            emit_wdma(1)
            emit_wsigns(1)
            srcs0 = emit_loads(0, chunks=CHUNKS[2:])
            emit_signs(0, srcs0)
            srcs = emit_loads(1)
            emit_signs(1, srcs)
            # interleave mt1 weight prep with img0's mt0 groups
            mt1_trans = [(cg, tri) for cg in range(2) for tri in range(3)]
            ots0 = {}
            for ci, h0 in enumerate(range(0, H, CHUNK)):
                ot = opool.tile([P, 2, CHUNK, W], mybir.dt.float32, name="ot")
                ots0[h0] = ot
                emit_mm_group(0, h0, 0, ot)
                if 3 <= ci and ci - 3 < len(mt1_trans):
                    cg, tri = mt1_trans[ci - 3]
                    emit_wtrans(1, cg, tri)
            # any trios the 7-group loop couldn't fit (ci reaches 6 -> 4
            # slots for 6 trios): emit the remainder before the mt1 groups
            for cg, tri in mt1_trans[4:]:
                emit_wtrans(1, cg, tri)
            for h0 in range(0, H, CHUNK):
                emit_mm_group(0, h0, 1, ots0[h0])
                nc.scalar.dma_start(
                    out=y_ap[0]
                    .rearrange("(mt c) h w -> c mt h w", mt=2)[
                        :, :, h0 : h0 + CHUNK, :
                    ],
                    in_=ots0[h0],
                )
            for img in range(1, IMG_PER_CORE):
                if img + 1 < IMG_PER_CORE:
                    srcs = emit_loads(img + 1)
                    emit_signs(img + 1, srcs)
                emit_mms(img)
    nc.compile()
    return nc


def kernel(x, weight, alpha, trace=False):
    global last_result
    x = np.ascontiguousarray(np.asarray(x, dtype=np.float32))
    weight = np.ascontiguousarray(np.asarray(weight, dtype=np.float32))
    alpha = np.ascontiguousarray(np.asarray(alpha, dtype=np.float32))

    nc = build_conv_kernel()
    in_maps = [
        {
            "x": np.ascontiguousarray(x[i * IMG_PER_CORE : (i + 1) * IMG_PER_CORE]),
            "w": weight,
            "alpha": alpha,
        }
        for i in range(N_CORES)
    ]
    res = run_bass_kernel_spmd(nc, in_maps, list(range(N_CORES)), trace=trace)
    last_result = res
    out = np.concatenate([res.results[i]["y"] for i in range(N_CORES)], axis=0)
    return out.astype(np.float32, copy=False)
